# revision 33
# baseline (speedup 1.0000x reference)
"""Grouped MoE MLP (64 experts) on 8 Trainium2 NeuronCores.

Strategy: expert parallelism. Each core owns 8 experts (size-sorted "snake"
assignment so every core gets the same per-slot padded token capacity and the
padding is tight). Both matmuls keep tokens as the moving operand:

    hT[f, t]   = w1t[e] (stationary, [h,f] tiles) @ xT (moving, [h, t])
    hT         = gelu(hT)                     (ScalarE, PSUM f32 -> SBUF bf16)
    outT[o, t] = w2[e] (stationary, [f,o] tiles) @ hT (moving, [f, t])

All DRAM tensors are laid out host-side so every DMA moves long contiguous
runs per partition (8-16KB for weights, ~4KB for activations):

    w1n[s, hi, fg, ko, fs] = w1[e_s, fg*512+fs, ko*128+hi]   (bf16)
    w2n[s, fi, og, fo, hs] = w2[e_s, fo*128+fi, og*512+hs]   (bf16)
    xn [hi, slot-block s: ko*Cj + t] = x_s[t, ko*128+hi]     (bf16)
    outn[oi, slot-block s: oo*Cj + t] = out_s[t, oo*128+oi]  (bf16)

Perf structure (measured ~249-254us max-core, ~234us tensor-active):
  - all input DMAs ride ONE HWDGE FIFO (nc.sync) in exact consumption
    order: completion order == issue order, so the bytes the PE needs
    next always get the full ~360GB/s;
  - 2MiB transfers amortize the ~0.7us per-transfer overhead that
    otherwise idles the SDMA engines ~20% (slot 0 stays fine-grained so
    the PE starts ~12us in);
  - deep pools (w1/w2 bufs=5 half-tiles, x bufs=3, ~202KB/partition)
    buffer ~2.5 slots of prefetch to absorb HBM arbitration jitter;
  - 14 dummy matmuls on a scratch tile warm the PE HAM clock gate
    (1.2GHz -> 2.4GHz) during the initial DMA wait;
  - PSUM accumulates f32 (4+4 banks), gelu on ScalarE PSUM->SBUF bf16,
    output written bf16 (halves store traffic) and upcast on host.
"""

import numpy as np

NCORES = 8
SLOTS = 8  # experts per core
NE = 64
H = 1024
F = 2048
T = 16384
P = 128
KO = H // P  # 8  k-tiles for mm1 (contraction over H)
FO = F // P  # 16 f-tiles (mm1 output tiles / mm2 contraction)
OO = H // P  # 8  output h-tiles for mm2
FS = 512  # w1 f-chunk width (DMA chunk granularity)
FG = F // FS  # 4 w1 chunks per slot
HS = 512  # w2 h-chunk width
OG = H // HS  # 2 w2 h-groups per slot
NMAX = 512  # max moving-operand length (one fp32 PSUM bank)

ACT_FN = "Gelu"  # overridable for CoreSim tests (Gelu not implemented there)

_prog_cache = {}


def _build_program(C):
    """Build the SPMD Bass program for per-slot token capacities C (len SLOTS)."""
    from contextlib import ExitStack

    import concourse.tile as tile
    from concourse import bacc, mybir
    from concourse.bass import MemorySpace

    bf16 = mybir.dt.bfloat16
    f32 = mybir.dt.float32
    CTOT = int(sum(C))

    nc = bacc.Bacc("TRN2", target_bir_lowering=False, debug=False, num_devices=NCORES)
    w1n_d = nc.dram_tensor("w1n", [SLOTS, P, FG * KO * FS], bf16, kind="ExternalInput").ap()
    w2n_d = nc.dram_tensor("w2n", [SLOTS, P, OG * FO * HS], bf16, kind="ExternalInput").ap()
    xn_d = nc.dram_tensor("xn", [P, KO * CTOT], bf16, kind="ExternalInput").ap()
    outn_d = nc.dram_tensor("outn", [P, OO * CTOT], bf16, kind="ExternalOutput").ap()

    with tile.TileContext(nc) as tc, ExitStack() as ctx:
        w1_pool = ctx.enter_context(tc.tile_pool(name="w1", bufs=5))
        w2_pool = ctx.enter_context(tc.tile_pool(name="w2", bufs=5))
        x_pool = ctx.enter_context(tc.tile_pool(name="x", bufs=4))
        h_pool = ctx.enter_context(tc.tile_pool(name="h", bufs=2))
        o_pool = ctx.enter_context(tc.tile_pool(name="o", bufs=2))
        ph_pool = ctx.enter_context(
            tc.tile_pool(name="ph", bufs=4, space=MemorySpace.PSUM)
        )
        po_pool = ctx.enter_context(
            tc.tile_pool(name="po", bufs=4, space=MemorySpace.PSUM)
        )

        # PE warmup: the HAM clock gate runs the PE at 1.2GHz until it has
        # been busy ~3.4us. Real matmuls can't start until the first x/w1
        # chunks land (~12us), so burn dummy matmuls on a scratch tile during
        # the DMA wait to enter the kernel at full 2.4GHz.
        warm_pool = ctx.enter_context(tc.tile_pool(name="warm", bufs=1))
        warm_sb = warm_pool.tile([P, 2 * P], bf16, tag="warm")
        nc.vector.memset(warm_sb, 0.0)
        ph_w = ph_pool.tile([P, NMAX], f32, tag="ph")
        for _ in range(50):  # ends ~11.9us, just before the first x/w1 land
            nc.tensor.matmul(
                ph_w[:, :P], warm_sb[:, :P], warm_sb[:, P:], start=True, stop=True
            )

        # Input DMAs go through nc.sync (one FIFO HWDGE queue) in exact
        # consumption order, slot by slot: FIFO completion order == issue
        # order, so the bytes the PE needs next always get the bandwidth.
        # Output stores ride the gpsimd SWDGE ring so they never block the
        # input stream.
        off = 0
        for j in range(SLOTS):
            Cj = int(C[j])
            x_sb = x_pool.tile([P, KO * Cj], bf16, tag="x")
            if j == 0:
                # halves (ko 0-3 / 4-7): mm1 fo0's early ko steps can begin
                # after the first 0.3MB instead of the full 0.6MB
                hk = KO // 2
                nc.sync.dma_start(
                    x_sb[:, : hk * Cj], xn_d[:, KO * off : KO * off + hk * Cj]
                )
                nc.sync.dma_start(
                    x_sb[:, hk * Cj :], xn_d[:, KO * off + hk * Cj : KO * (off + Cj)]
                )
            else:
                nc.sync.dma_start(x_sb, xn_d[:, KO * off : KO * (off + Cj)])
            # One FIFO ring (sync), consumption order. Big transfers amortize
            # the per-transfer fixed overhead (~0.7us) that idles the SDMA
            # engines ~20%+ between 1MiB transfers; slot 0 stays fine-grained
            # so the PE can start as early as possible.
            w1h = KO * FS  # half-tile columns (2 fg chunks)
            w1_sba = w1_pool.tile([P, 2 * w1h], bf16, tag="w1")
            w1_sbb = w1_pool.tile([P, 2 * w1h], bf16, tag="w1")
            if j == 0:
                hh = w1h // 2
                for hc in range(2 * FG):  # 0.5MiB chunks: earliest PE start
                    t = w1_sba if hc < 4 else w1_sbb
                    c0 = (hc % 4) * hh
                    nc.sync.dma_start(
                        t[:, c0 : c0 + hh], w1n_d[j, :, hc * hh : (hc + 1) * hh]
                    )
            else:
                nc.sync.dma_start(w1_sba, w1n_d[j, :, 0 : 2 * w1h])
                nc.sync.dma_start(w1_sbb, w1n_d[j, :, 2 * w1h : 4 * w1h])
            w2_sba = w2_pool.tile([P, FO * HS], bf16, tag="w2")
            w2_sbb = w2_pool.tile([P, FO * HS], bf16, tag="w2")
            for og, t in ((0, w2_sba), (1, w2_sbb)):  # 2MiB transfers
                c0 = og * FO * HS
                if j == 0:
                    fh = FO // 2
                    for wc in range(2):  # 1MiB chunks on slot 0
                        nc.sync.dma_start(
                            t[:, wc * fh * HS : (wc + 1) * fh * HS],
                            w2n_d[j, :, c0 + wc * fh * HS : c0 + (wc + 1) * fh * HS],
                        )
                else:
                    nc.sync.dma_start(t, w2n_d[j, :, c0 : c0 + FO * HS])
            o_sb = o_pool.tile([P, OO * Cj], bf16, tag="o")

            for nb in range(0, Cj, NMAX):
                NB = min(NMAX, Cj - nb)
                h_sb = h_pool.tile([P, FO * NB], bf16, tag="h")
                for fo in range(FO):
                    fg, fs = fo // 4, (fo % 4) * P
                    w1t, fgl = (w1_sba, fg) if fg < 2 else (w1_sbb, fg - 2)
                    ph = ph_pool.tile([P, NMAX], f32, tag="ph")
                    for ko in range(KO):
                        c0 = (fgl * KO + ko) * FS + fs
                        nc.tensor.matmul(
                            ph[:, :NB],
                            w1t[:, c0 : c0 + P],
                            x_sb[:, ko * Cj + nb : ko * Cj + nb + NB],
                            start=(ko == 0),
                            stop=(ko == KO - 1),
                        )
                    nc.scalar.activation(
                        h_sb[:, fo * NB : (fo + 1) * NB],
                        ph[:, :NB],
                        getattr(mybir.ActivationFunctionType, ACT_FN),
                    )
                for oo in range(OO):
                    og, hs = oo // 4, (oo % 4) * P
                    w2t = w2_sba if og == 0 else w2_sbb
                    po = po_pool.tile([P, NMAX], f32, tag="po")
                    for fo in range(FO):
                        nc.tensor.matmul(
                            po[:, :NB],
                            w2t[:, fo * HS + hs : fo * HS + hs + P],
                            h_sb[:, fo * NB : fo * NB + NB],
                            start=(fo == 0),
                            stop=(fo == FO - 1),
                        )
                    nc.vector.tensor_copy(
                        o_sb[:, oo * Cj + nb : oo * Cj + nb + NB], po[:, :NB]
                    )
            # split the store so earlier oo tiles fly while later ones compute;
            # last slot goes in quarters on the (empty by then) sync HWDGE
            # ring, whose fixed cost is lower than gpsimd SWDGE, to cut the
            # kernel tail.
            if j < SLOTS - 1:
                hoo = OO // 2
                nc.gpsimd.dma_start(
                    outn_d[:, OO * off : OO * off + hoo * Cj], o_sb[:, : hoo * Cj]
                )
                nc.gpsimd.dma_start(
                    outn_d[:, OO * off + hoo * Cj : OO * (off + Cj)],
                    o_sb[:, hoo * Cj :],
                )
            else:
                qoo = OO // 4
                for q in range(4):
                    nc.sync.dma_start(
                        outn_d[
                            :, OO * off + q * qoo * Cj : OO * off + (q + 1) * qoo * Cj
                        ],
                        o_sb[:, q * qoo * Cj : (q + 1) * qoo * Cj],
                    )
            off += Cj

    nc.compile()
    return nc


def _get_program(C):
    key = tuple(int(c) for c in C)
    if key not in _prog_cache:
        _prog_cache[key] = _build_program(key)
    return _prog_cache[key]


def plan(sizes):
    """Expert->core/slot assignment + slot capacities from token counts."""
    sizes = np.asarray(sizes, np.int64)
    assert sizes.shape == (NE,) and sizes.sum() == T
    order = np.argsort(-sizes, kind="stable")  # descending
    # expert_of[core][slot]
    expert_of = [[int(order[s * NCORES + c]) for s in range(SLOTS)] for c in range(NCORES)]
    C = []
    for s in range(SLOTS):
        m = max(int(sizes[order[s * NCORES + c]]) for c in range(NCORES))
        # multiple-of-4 keeps every SBUF column slice 8B-aligned (bf16);
        # finer rounding (tried 2) degrades the PE moving-operand stream.
        C.append(max(4, -(-m // 4) * 4))
    offs = np.concatenate([[0], np.cumsum(C)]).astype(np.int64)
    return expert_of, C, offs


def prepare_inputs(x, w1, w2, sizes, expert_of, C, offs):
    """Host-side shard/pad/transpose/cast. Returns per-core input maps."""
    import ml_dtypes

    bf16 = ml_dtypes.bfloat16
    x = np.asarray(x, np.float32)
    tok_offs = np.concatenate([[0], np.cumsum(sizes)]).astype(np.int64)
    w1_bf = np.asarray(w1, np.float32).astype(bf16)  # [NE, F, H]
    w2_bf = np.asarray(w2, np.float32).astype(bf16)  # [NE, F, H]
    CTOT = int(sum(C))

    in_maps = []
    for c in range(NCORES):
        experts = expert_of[c]
        # w1n[s, hi, fg, ko, fs] = w1[e, fg*FS+fs, ko*P+hi]
        w1n = np.ascontiguousarray(
            w1_bf[experts].reshape(SLOTS, FG, FS, KO, P).transpose(0, 4, 1, 3, 2)
        ).reshape(SLOTS, P, FG * KO * FS)
        # w2n[s, fi, og, fo, hs] = w2[e, fo*P+fi, og*HS+hs]
        w2n = np.ascontiguousarray(
            w2_bf[experts].reshape(SLOTS, FO, P, OG, HS).transpose(0, 2, 3, 1, 4)
        ).reshape(SLOTS, P, OG * FO * HS)
        xn = np.zeros((P, KO * CTOT), bf16)
        for s, e in enumerate(experts):
            n = int(sizes[e])
            Cj = int(C[s])
            xs = np.zeros((Cj, KO, P), np.float32)
            xs[:n] = x[tok_offs[e] : tok_offs[e] + n].reshape(n, KO, P)
            # xn block: [P, KO, Cj]
            xn[:, KO * offs[s] : KO * (offs[s] + Cj)] = (
                xs.transpose(2, 1, 0).reshape(P, KO * Cj).astype(bf16)
            )
        in_maps.append({"w1n": w1n, "w2n": w2n, "xn": xn})
    return in_maps


def scatter_output(results, sizes, expert_of, C, offs):
    """Gather per-core outputs back into the full [T, H] f32 output."""
    tok_offs = np.concatenate([[0], np.cumsum(sizes)]).astype(np.int64)
    out = np.empty((T, H), np.float32)
    for c in range(NCORES):
        outn = np.asarray(results[c]["outn"])  # [P, OO*CTOT] bf16
        for s, e in enumerate(expert_of[c]):
            n = int(sizes[e])
            Cj = int(C[s])
            blk = outn[:, OO * offs[s] : OO * (offs[s] + Cj)].reshape(P, OO, Cj)
            # out[t, oo*P+oi] = blk[oi, oo, t]
            out[tok_offs[e] : tok_offs[e] + n] = (
                blk[:, :, :n].transpose(2, 1, 0).reshape(n, H).astype(np.float32)
            )
    return out


def kernel(x, w1, w2, tokens_per_expert):
    from concourse import bass2jax

    sizes = np.asarray(tokens_per_expert, np.int64)
    expert_of, C, offs = plan(sizes)
    nc = _get_program(C)
    in_maps = prepare_inputs(x, w1, w2, sizes, expert_of, C, offs)
    results = bass2jax.run_bass_via_pjrt(nc, in_maps, n_cores=NCORES)
    return scatter_output(results, sizes, expert_of, C, offs)
